# revision 30
# baseline (speedup 1.0000x reference)
"""Trainium2 Bass kernel for nn_BoxSeg_24043226923460 (nms_detection).

Reference computation:
  - pairwise IoU between pred_boxes [100,4] and gt_boxes [20,4]
  - sequential greedy match over candidates in descending-score order
    (faithful to the reference's `gtm[j] <= 0` availability rule)
  - gather: gt_bitmasks[j] = pred_masks[gtm[j]] for matched j else zeros,
    plus matched mask_score / pred_scores vectors.

Device strategy (SPMD over 8 cores):
  - Every core runs the (tiny) match replicated: IoU on 100 partitions,
    exact argsort-by-score priority via a comparison-matrix matmul, and a
    round-based parallel greedy that is provably identical to the
    sequential scan: each round every active candidate bids its argmax
    available GT, and we commit exactly the candidates ranked above the
    highest-priority losing bidder (<= 22 rounds worst case, ~3 typical,
    with a register-If checkpoint after each early round).
  - The heavy data movement is sharded over H: core c owns rows
    [c*100, (c+1)*100) of every mask.  The host appends a zeros page to
    the mask slab (index 100); each of the 20 outputs is its own DRAM
    tensor (separate tensors keep Tile from serializing the copies).
    Because gt_boxes are perturbed copies of preds[:20], gtm[j] == j for
    most GTs, so 20 static speculative DMAs copy mask j -> output j at
    kernel start (hidden under the match), and a post-match repair wave
    of cond-predicated dynamic-offset DRAM->DRAM DMAs rewrites only the
    slots where speculation missed (zeros page for unmatched GTs).
"""

import numpy as np

M, N, H, W = 100, 20, 800, 800
NCORES = 8
RS = H // NCORES  # 100 rows per core
IOU_THR = 0.5
SCORE_THR = 0.05
BIG = 1.0e9
NEG_TEST = -1.0e8  # "is a real iou value" threshold (valid iou >= 0 > NEG_TEST)

# rounds before checkpointing starts=2 (wave A issues after round 2);
# then one checkpoint per early round, groups later.  Total 22.
ROUND_PLAN = [2, 1, 1, 1, 1, 2, 4, 10]

_CACHE = {}


def _build():
    import concourse.bass as bass
    import concourse.mybir as mybir
    from concourse import bacc
    from concourse import bass_isa
    from concourse.tile import TileContext

    f32 = mybir.dt.float32
    bf16 = mybir.dt.bfloat16
    i32 = mybir.dt.int32
    u32 = mybir.dt.uint32
    AX = mybir.AxisListType
    OP = mybir.AluOpType
    ET = mybir.EngineType
    RED = bass_isa.ReduceOp

    nc = bacc.Bacc("TRN2", target_bir_lowering=False, debug=False)

    # ---- DRAM I/O ----------------------------------------------------
    slab = nc.dram_tensor("slab", [M + 1, RS, W], f32, kind="ExternalInput")
    pb_d = nc.dram_tensor("pred_boxes", [M, 4], f32, kind="ExternalInput")
    ps_d = nc.dram_tensor("pred_scores", [1, M], f32, kind="ExternalInput")
    ms_d = nc.dram_tensor("mask_score", [1, M], f32, kind="ExternalInput")
    gt_d = nc.dram_tensor("gt_boxes", [1, 4 * N], f32, kind="ExternalInput")

    outm = [
        nc.dram_tensor(f"out_m_{j}", [RS, W], f32, kind="ExternalOutput")
        for j in range(N)
    ]
    outf = nc.dram_tensor("out_flags", [1, N], f32, kind="ExternalOutput")
    outs = nc.dram_tensor("out_scores", [1, N], f32, kind="ExternalOutput")

    import contextlib

    with TileContext(nc) as tc, contextlib.ExitStack() as pools:
        sb = pools.enter_context(tc.tile_pool(name="sb", bufs=1))
        ps = pools.enter_context(tc.tile_pool(name="ps", bufs=3, space="PSUM"))
        rnd = pools.enter_context(tc.tile_pool(name="rnd", bufs=2))

        V = nc.vector
        T = nc.tensor
        G = nc.gpsimd

        def mask_wave(comb_i_tile, wave, cond_lt):
            """20 dynamic-offset DRAM->DRAM mask copies, split over the two
            HWDGE engines, indices fetched with one multi-register load per
            engine.  Bounds claim M even though skipped values exceed it:
            the cond multiply zeroes the offset before use, and the
            intermediates stay far below 2^31 for every real value."""
            for half, (eng, et) in enumerate(
                    [(nc.scalar, ET.Activation), (nc.sync, ET.SP)]):
                regs = [nc.alloc_register(et, f"dm_{wave}_{half}_{k}")
                        for k in range(N // 2)]
                j0 = half * (N // 2)
                eng.reg_load(regs, comb_i_tile[0:1, j0:j0 + N // 2])
                for k, r in enumerate(regs):
                    j = j0 + k
                    sv = nc.s_assert_within(
                        bass.RuntimeValue(r), 0, M, skip_runtime_assert=True)
                    eng.dma_start(out=outm[j][:, :], in_=slab[sv],
                                  cond=sv < cond_lt, cond_hint=True)

        # ---- load small inputs (spread across DMA engines) -----------
        pb = sb.tile([M, 4], f32)
        nc.sync.dma_start(out=pb[:], in_=pb_d[:, :])
        sc_row = sb.tile([1, M], f32)
        nc.scalar.dma_start(out=sc_row[:], in_=ps_d[:, :])
        ms_row = sb.tile([1, M], f32)
        nc.scalar.dma_start(out=ms_row[:], in_=ms_d[:, :])
        gt_row = sb.tile([1, 4 * N], f32)  # gt-major: [g0(x1 y1 x2 y2), g1(...), ...]
        nc.sync.dma_start(out=gt_row[:], in_=gt_d[:, :])

        # ---- speculative prefetch: out[j] <- mask j (static DMAs) -----
        # gt_boxes are perturbed copies of preds[:N], so gtm[j] == j for
        # most GTs; stream those copies during the match, repair the rest
        # in the post-match cond wave below.
        for j in range(N):
            eng = nc.scalar if j % 2 == 0 else nc.sync
            eng.dma_start(out=outm[j][:, :], in_=slab[j])

        # ---- static constants ----------------------------------------
        iotaF_i = sb.tile([M, M], i32)   # [p, f] = f
        G.iota(iotaF_i[:], pattern=[[1, M]], base=0, channel_multiplier=0)
        iotaP_i = sb.tile([M, 1], i32)   # [p, 0] = p
        G.iota(iotaP_i[:], pattern=[[0, 1]], base=0, channel_multiplier=1)
        iota20b_u = sb.tile([M, N], u32)  # [p, j] = j
        G.iota(iota20b_u[:], pattern=[[1, N]], base=0, channel_multiplier=0,
               allow_small_or_imprecise_dtypes=True)
        iota20b = sb.tile([M, N], f32)
        V.tensor_copy(iota20b[:], iota20b_u[:])

        iotaF = sb.tile([M, M], f32)
        V.tensor_copy(iotaF[:], iotaF_i[:])
        iotaP = sb.tile([M, 1], f32)
        V.tensor_copy(iotaP[:], iotaP_i[:])
        iotaP_bf = sb.tile([M, 1], bf16)
        V.tensor_copy(iotaP_bf[:], iotaP_i[:])

        ones_row = sb.tile([1, M], f32)
        V.memset(ones_row[:], 1.0)
        ones_row_bf = sb.tile([1, M], bf16)
        V.memset(ones_row_bf[:], 1.0)
        ones_col = sb.tile([M, 1], f32)
        V.memset(ones_col[:], 1.0)
        ones_col_bf = sb.tile([M, 1], bf16)
        V.memset(ones_col_bf[:], 1.0)

        # identity (for PE transpose) and "f > p" matrix, from iotas
        ident = sb.tile([M, M], f32)
        V.tensor_scalar(ident[:], iotaF[:], iotaP[:], None, OP.is_equal)
        ltmat = sb.tile([M, M], f32)  # [p, f] = (f > p)
        V.tensor_scalar(ltmat[:], iotaF[:], iotaP[:], None, OP.is_gt)

        # score column [100,1] via PE transpose, and mask_score column
        sc_col = sb.tile([M, 1], f32)
        sccol_ps = ps.tile([M, 1], f32, space="PSUM", tag="ps")
        T.transpose(out=sccol_ps[:], in_=sc_row[:], identity=ident[:1, :1])
        V.tensor_copy(sc_col[:], sccol_ps[:])
        # ---- IoU [100, 20] -------------------------------------------
        # component-major gt row: [x1*20, y1*20, x2*20, y2*20, area2*20]
        crow = sb.tile([1, 5 * N], f32)
        for c in range(4):
            V.tensor_copy(crow[0:1, c * N:(c + 1) * N], gt_row[0:1, c::4])
        aw = sb.tile([1, N], f32)
        V.tensor_sub(aw[:], crow[0:1, 2 * N:3 * N], crow[0:1, 0:N])
        ah = sb.tile([1, N], f32)
        V.tensor_sub(ah[:], crow[0:1, 3 * N:4 * N], crow[0:1, N:2 * N])
        V.tensor_mul(crow[0:1, 4 * N:5 * N], aw[:], ah[:])

        gtb_ps = ps.tile([M, 5 * N], f32, space="PSUM", tag="ps")
        T.matmul(out=gtb_ps[:], lhsT=ones_row[:], rhs=crow[:], start=True, stop=True)
        gx1 = gtb_ps[:, 0:N]
        gy1 = gtb_ps[:, N:2 * N]
        gx2 = gtb_ps[:, 2 * N:3 * N]
        gy2 = gtb_ps[:, 3 * N:4 * N]
        ga = gtb_ps[:, 4 * N:5 * N]

        ltx = sb.tile([M, N], f32)
        V.tensor_scalar(ltx[:], gx1, pb[:, 0:1], None, OP.max)
        lty = sb.tile([M, N], f32)
        V.tensor_scalar(lty[:], gy1, pb[:, 1:2], None, OP.max)
        rbx = sb.tile([M, N], f32)
        V.tensor_scalar(rbx[:], gx2, pb[:, 2:3], None, OP.min)
        rby = sb.tile([M, N], f32)
        V.tensor_scalar(rby[:], gy2, pb[:, 3:4], None, OP.min)
        V.tensor_sub(ltx[:], rbx[:], ltx[:])                       # w
        V.tensor_scalar(ltx[:], ltx[:], 0.0, None, OP.max)
        V.tensor_sub(lty[:], rby[:], lty[:])                       # h
        V.tensor_scalar(lty[:], lty[:], 0.0, None, OP.max)
        inter = sb.tile([M, N], f32)
        V.tensor_mul(inter[:], ltx[:], lty[:])

        ar1a = sb.tile([M, 1], f32)
        V.tensor_sub(ar1a[:], pb[:, 2:3], pb[:, 0:1])
        ar1b = sb.tile([M, 1], f32)
        V.tensor_sub(ar1b[:], pb[:, 3:4], pb[:, 1:2])
        V.tensor_mul(ar1a[:], ar1a[:], ar1b[:])                    # area1 [100,1]

        union = sb.tile([M, N], f32)
        V.tensor_sub(union[:], ga, inter[:])                       # area2 - inter
        V.tensor_scalar(union[:], union[:], ar1a[:], 1e-6, OP.add, OP.max)
        rec = sb.tile([M, N], f32)
        V.reciprocal(rec[:], union[:])
        iou = sb.tile([M, N], f32)
        V.tensor_mul(iou[:], inter[:], rec[:])

        # masked score matrix: iou where (iou >= THR and score ok) else ~ -BIG
        scok = sb.tile([M, 1], f32)
        V.tensor_scalar(scok[:], sc_col[:], SCORE_THR, None, OP.is_ge)
        gate = sb.tile([M, N], f32)
        V.tensor_scalar(gate[:], iou[:], IOU_THR, scok[:], OP.is_ge, OP.mult)
        V.tensor_scalar(gate[:], gate[:], BIG, BIG, OP.mult, OP.subtract)
        masked_s = sb.tile([M, N], f32)
        V.tensor_add(masked_s[:], iou[:], gate[:])
        curmask = sb.tile([M, N], f32)

        # round-1 bids: issue as soon as masked_s exists so they overlap the
        # PE half of the priority chain below
        maxv8 = sb.tile([M, 8], f32)
        idx8 = sb.tile([M, 8], u32)
        V.max(out=maxv8[:], in_=masked_s[:])
        V.max_index(idx8[:], maxv8[:], masked_s[:])

        # ---- exact priority (stable argsort of -scores) ---------------
        # rank[i] = #{j: s[j] > s[i]} + #{j < i: s[j] == s[i]}; prio = 100 - rank
        srow_ps = ps.tile([M, M], f32, space="PSUM", tag="ps")
        T.matmul(out=srow_ps[:], lhsT=ones_row[:], rhs=sc_row[:], start=True, stop=True)
        a1 = sb.tile([M, M], f32)
        V.tensor_scalar(a1[:], srow_ps[:], sc_col[:], None, OP.is_lt)   # s[f] < s[p]
        aeq = sb.tile([M, M], f32)
        V.tensor_scalar(aeq[:], srow_ps[:], sc_col[:], None, OP.is_equal)
        V.tensor_mul(aeq[:], aeq[:], ltmat[:])          # (s[f]==s[p]) & (f > p)
        V.tensor_add(a1[:], a1[:], aeq[:])              # A[p, f] (p beats f)
        rank_ps = ps.tile([1, M], f32, space="PSUM", tag="ps")    # rank[f] = sum_p A[p, f]
        T.matmul(out=rank_ps[:], lhsT=ones_col[:], rhs=a1[:], start=True, stop=True)
        rank_row = sb.tile([1, M], f32)
        V.tensor_copy(rank_row[:], rank_ps[:])
        rankc_ps = ps.tile([M, 1], f32, space="PSUM", tag="ps")
        T.transpose(out=rankc_ps[:], in_=rank_row[:], identity=ident[:1, :1])
        prio = sb.tile([M, 1], f32)                     # prio = 100 - rank in [1, 100]
        V.tensor_scalar(prio[:], rankc_ps[:], -1.0, float(M), OP.mult, OP.add)

        # ---- match state ---------------------------------------------
        gtm = sb.tile([1, N], f32)
        V.memset(gtm[:], -1.0)
        actp = sb.tile([M, 1], f32)      # prio if active else 0
        # init + round-1 deactivation fused: actp = prio * (best bid valid)
        V.tensor_scalar(actp[:], maxv8[:, 0:1], NEG_TEST, prio[:],
                        OP.is_gt, OP.mult)
        actcnt_i = sb.tile([1, 1], i32)
        combB_i = sb.tile([1, N], i32)

        def peek(src):
            # bids for the NEXT round + deactivate candidates w/o valid bids
            V.max(out=maxv8[:], in_=src[:])
            V.max_index(idx8[:], maxv8[:], src[:])
            V.tensor_scalar(actp[:], maxv8[:, 0:1], NEG_TEST, actp[:],
                            OP.is_gt, OP.mult)

        def commit_round():
            # bid matrix scaled by active priority
            mf = rnd.tile([M, 1], f32, tag="mf")
            V.tensor_copy(mf[:], idx8[:, 0:1])
            bp = rnd.tile([M, N], f32, tag="bp")
            V.tensor_scalar(bp[:], iota20b[:], mf[:], actp[:],
                            OP.is_equal, OP.mult)
            actp2 = rnd.tile([M, 1], f32, tag="actp2")
            V.tensor_mul(actp2[:], actp[:], actp[:])
            # per-GT winner priority broadcast to all partitions
            lb = rnd.tile([M, N], f32, tag="lb")
            G.partition_all_reduce(lb[:], bp[:], channels=M, reduce_op=RED.max)
            # per-candidate: did a higher-priority bidder take my GT?
            lt = rnd.tile([M, N], f32, tag="lt")
            V.tensor_mul(lt[:], lb[:], bp[:])       # actp * winner_prio at own bid
            lo = rnd.tile([M, 1], f32, tag="lo")
            V.tensor_reduce(lo[:], lt[:], AX.X, OP.max)
            loserp = rnd.tile([M, 1], f32, tag="loserp")
            V.tensor_scalar(loserp[:], lo[:], actp2[:], actp[:],
                            OP.is_gt, OP.mult)       # prio of losing bidders
            stopb = rnd.tile([M, 1], f32, tag="stopb")
            G.partition_all_reduce(stopb[:], loserp[:], channels=M,
                                   reduce_op=RED.max)
            # commit candidates strictly above the best loser
            commit = rnd.tile([M, 1], f32, tag="commit")
            V.tensor_tensor(commit[:], actp[:], stopb[:], OP.is_gt)
            V.scalar_tensor_tensor(actp[:], actp[:], stopb[:], actp[:],
                                   OP.is_le, OP.mult)
            wc = rnd.tile([M, N], bf16, tag="wc")
            V.tensor_scalar(wc[:], iota20b[:], mf[:], commit[:],
                            OP.is_equal, OP.mult)
            wrote_ps = ps.tile([1, N], f32, space="PSUM", tag="ps")
            T.matmul(out=wrote_ps[:], lhsT=ones_col_bf[:], rhs=wc[:],
                     start=True, stop=True)
            widx_ps = ps.tile([1, N], f32, space="PSUM", tag="ps")
            T.matmul(out=widx_ps[:], lhsT=iotaP_bf[:], rhs=wc[:],
                     start=True, stop=True)
            # gtm += wrote * (widx - gtm)
            t1 = rnd.tile([1, N], f32, tag="t1")
            V.tensor_sub(t1[:], widx_ps[0:1, :], gtm[:])
            V.tensor_mul(t1[:], t1[:], wrote_ps[0:1, :])
            V.tensor_add(gtm[:], gtm[:], t1[:])
            # rebuild curmask with -BIG on locked GTs (gtm >= 1)
            pen = rnd.tile([1, N], bf16, tag="pen")
            V.tensor_scalar(pen[:], gtm[:], 0.5, -BIG, OP.is_gt, OP.mult)
            penb_ps = ps.tile([M, N], f32, space="PSUM", tag="ps")
            T.matmul(out=penb_ps[:], lhsT=ones_row_bf[:], rhs=pen[:],
                     start=True, stop=True)
            V.tensor_add(curmask[:], masked_s[:], penb_ps[:])

        ck_stack = contextlib.ExitStack()

        def checkpoint(tag):
            acnt_ps = ps.tile([1, 1], f32, space="PSUM", tag="ps")
            T.matmul(out=acnt_ps[:], lhsT=actp[:], rhs=ones_col[:],
                     start=True, stop=True)
            V.tensor_copy(actcnt_i[:], acnt_ps[:])
            regs = nc.alloc_registers(
                f"ck_{tag}", bass.OrderedSet([ET.DVE, ET.PE, ET.Pool]))
            for reg in regs:
                nc.reg_load(reg, actcnt_i[0:1, 0:1])
            ck_stack.enter_context(tc.If(nc.snap(regs) > 0))

        for gi, g in enumerate(ROUND_PLAN):
            if gi > 0:
                checkpoint(gi)
            for _ in range(g):
                commit_round()
                peek(curmask)
        ck_stack.close()

        # ---- post-match: indices + small outputs ---------------------
        mt = sb.tile([1, N], f32)
        V.tensor_scalar(mt[:], gtm[:], -0.5, None, OP.is_gt)       # matched
        comb = sb.tile([1, N], f32)
        V.tensor_scalar(comb[:], mt[:], -float(M), float(M), OP.mult, OP.add)
        icl = sb.tile([1, N], f32)
        V.tensor_scalar(icl[:], gtm[:], 0.0, None, OP.max)
        V.tensor_add(comb[:], comb[:], icl[:])   # idx (matched) / 100 (unmatched)
        # repair wave: skip GTs where the speculative prefetch was right
        spec_ok = sb.tile([1, N], f32)
        V.tensor_tensor(spec_ok[:], gtm[:], iota20b[0:1, :], OP.is_equal)
        V.scalar_tensor_tensor(comb[:], spec_ok[:], 1000.0, comb[:],
                               OP.mult, OP.add)
        V.tensor_copy(combB_i[:], comb[:])

        mask_wave(combB_i, "B", M + 1)

        # small outputs: scores/flags of matched candidates via one-hot matmul
        ms_col = sb.tile([M, 1], f32)
        mscol_ps = ps.tile([M, 1], f32, space="PSUM", tag="ps")
        T.transpose(out=mscol_ps[:], in_=ms_row[:], identity=ident[:1, :1])
        V.tensor_copy(ms_col[:], mscol_ps[:])
        gtmb_ps = ps.tile([M, N], f32, space="PSUM", tag="ps")
        T.matmul(out=gtmb_ps[:], lhsT=ones_row[:], rhs=gtm[:], start=True, stop=True)
        wsel = sb.tile([M, N], f32)
        V.tensor_scalar(wsel[:], gtmb_ps[:], iotaP[:], None, OP.is_equal)
        sm2 = sb.tile([M, 2], f32)
        V.tensor_copy(sm2[:, 0:1], sc_col[:])
        V.tensor_copy(sm2[:, 1:2], ms_col[:])
        o2_ps = ps.tile([2, N], f32, space="PSUM", tag="ps")
        T.matmul(out=o2_ps[:], lhsT=sm2[:], rhs=wsel[:], start=True, stop=True)
        o2 = sb.tile([2, N], f32)
        V.tensor_copy(o2[:], o2_ps[:])
        nc.sync.dma_start(out=outs[:, :], in_=o2[0:1, :])
        nc.sync.dma_start(out=outf[:, :], in_=o2[1:2, :])

    nc.compile()
    return nc


def _get_nc():
    if "nc" not in _CACHE:
        _CACHE["nc"] = _build()
    return _CACHE["nc"]


def kernel(pred_boxes, pred_scores, mask_score, pred_masks, gt_boxes):
    from concourse.bass_utils import run_bass_kernel_spmd

    nc = _get_nc()

    pred_boxes = np.ascontiguousarray(pred_boxes, dtype=np.float32)
    pred_scores = np.ascontiguousarray(pred_scores, dtype=np.float32).reshape(1, M)
    mask_score = np.ascontiguousarray(mask_score, dtype=np.float32).reshape(1, M)
    gt_boxes = np.ascontiguousarray(gt_boxes, dtype=np.float32).reshape(1, 4 * N)
    pred_masks = np.asarray(pred_masks, dtype=np.float32)

    in_maps = []
    for c in range(NCORES):
        sl = np.empty((M + 1, RS, W), dtype=np.float32)
        sl[:M] = pred_masks[:, c * RS:(c + 1) * RS, :]
        sl[M] = 0.0
        in_maps.append({
            "slab": sl,
            "pred_boxes": pred_boxes,
            "pred_scores": pred_scores,
            "mask_score": mask_score,
            "gt_boxes": gt_boxes,
        })

    res = run_bass_kernel_spmd(nc, in_maps, list(range(NCORES)))
    _CACHE["last_res"] = res
    results = res.results

    gt_bitmasks = np.empty((N, H, W), dtype=np.float32)
    for j in range(N):
        for c in range(NCORES):
            gt_bitmasks[j, c * RS:(c + 1) * RS, :] = results[c][f"out_m_{j}"]
    gt_masks_flags = results[0]["out_flags"].reshape(N)
    out_pred_scores = results[0]["out_scores"].reshape(N)
    return gt_bitmasks, gt_masks_flags, out_pred_scores


# revision 31
# speedup vs baseline: 1.0219x; 1.0219x over previous
"""Trainium2 Bass kernel for nn_BoxSeg_24043226923460 (nms_detection).

Reference computation:
  - pairwise IoU between pred_boxes [100,4] and gt_boxes [20,4]
  - sequential greedy match over candidates in descending-score order
    (faithful to the reference's `gtm[j] <= 0` availability rule)
  - gather: gt_bitmasks[j] = pred_masks[gtm[j]] for matched j else zeros,
    plus matched mask_score / pred_scores vectors.

Device strategy (SPMD over 8 cores):
  - Every core runs the (tiny) match replicated: IoU on 100 partitions,
    exact argsort-by-score priority via a comparison-matrix matmul, and a
    round-based parallel greedy that is provably identical to the
    sequential scan: each round every active candidate bids its argmax
    available GT, and we commit exactly the candidates ranked above the
    highest-priority losing bidder (<= 22 rounds worst case, ~3 typical,
    with a register-If checkpoint after each early round).
  - The heavy data movement is sharded over H: core c owns rows
    [c*100, (c+1)*100) of every mask.  The host appends a zeros page to
    the mask slab (index 100); each of the 20 outputs is its own DRAM
    tensor (separate tensors keep Tile from serializing the copies).
    Because gt_boxes are perturbed copies of preds[:20], gtm[j] == j for
    most GTs, so 20 static speculative DMAs copy mask j -> output j at
    kernel start (hidden under the match), and a post-match repair wave
    of cond-predicated dynamic-offset DRAM->DRAM DMAs rewrites only the
    slots where speculation missed (zeros page for unmatched GTs).
"""

import numpy as np

M, N, H, W = 100, 20, 800, 800
NCORES = 8
RS = H // NCORES  # 100 rows per core
IOU_THR = 0.5
SCORE_THR = 0.05
BIG = 1.0e9
NEG_TEST = -1.0e8  # "is a real iou value" threshold (valid iou >= 0 > NEG_TEST)

# rounds before checkpointing starts=2 (wave A issues after round 2);
# then one checkpoint per early round, groups later.  Total 22.
ROUND_PLAN = [2, 1, 1, 1, 1, 2, 4, 10]

_CACHE = {}


def _build():
    import concourse.bass as bass
    import concourse.mybir as mybir
    from concourse import bacc
    from concourse import bass_isa
    from concourse.tile import TileContext

    f32 = mybir.dt.float32
    bf16 = mybir.dt.bfloat16
    i32 = mybir.dt.int32
    u32 = mybir.dt.uint32
    AX = mybir.AxisListType
    OP = mybir.AluOpType
    ET = mybir.EngineType
    RED = bass_isa.ReduceOp

    nc = bacc.Bacc("TRN2", target_bir_lowering=False, debug=False)

    # ---- DRAM I/O ----------------------------------------------------
    slab = nc.dram_tensor("slab", [M + 1, RS, W], f32, kind="ExternalInput")
    pb_d = nc.dram_tensor("pred_boxes", [M, 4], f32, kind="ExternalInput")
    ps_d = nc.dram_tensor("pred_scores", [1, M], f32, kind="ExternalInput")
    ms_d = nc.dram_tensor("mask_score", [1, M], f32, kind="ExternalInput")
    gt_d = nc.dram_tensor("gt_boxes", [1, 4 * N], f32, kind="ExternalInput")

    outm = [
        nc.dram_tensor(f"out_m_{j}", [RS, W], f32, kind="ExternalOutput")
        for j in range(N)
    ]
    outf = nc.dram_tensor("out_flags", [1, N], f32, kind="ExternalOutput")
    outs = nc.dram_tensor("out_scores", [1, N], f32, kind="ExternalOutput")

    import contextlib

    with TileContext(nc) as tc, contextlib.ExitStack() as pools:
        sb = pools.enter_context(tc.tile_pool(name="sb", bufs=1))
        ps = pools.enter_context(tc.tile_pool(name="ps", bufs=3, space="PSUM"))
        rnd = pools.enter_context(tc.tile_pool(name="rnd", bufs=2))

        V = nc.vector
        T = nc.tensor
        G = nc.gpsimd

        def mask_wave(comb_i_tile, wave, cond_lt):
            """20 dynamic-offset DRAM->DRAM mask copies, split over the two
            HWDGE engines, indices fetched with one multi-register load per
            engine.  Bounds claim M even though skipped values exceed it:
            the cond multiply zeroes the offset before use, and the
            intermediates stay far below 2^31 for every real value."""
            for half, (eng, et) in enumerate(
                    [(nc.scalar, ET.Activation), (nc.sync, ET.SP)]):
                regs = [nc.alloc_register(et, f"dm_{wave}_{half}_{k}")
                        for k in range(N // 2)]
                j0 = half * (N // 2)
                eng.reg_load(regs, comb_i_tile[0:1, j0:j0 + N // 2])
                for k, r in enumerate(regs):
                    j = j0 + k
                    sv = nc.s_assert_within(
                        bass.RuntimeValue(r), 0, M, skip_runtime_assert=True)
                    eng.dma_start(out=outm[j][:, :], in_=slab[sv],
                                  cond=sv < cond_lt, cond_hint=True)

        # ---- load small inputs (spread across DMA engines) -----------
        pb = sb.tile([M, 4], f32)
        nc.sync.dma_start(out=pb[:], in_=pb_d[:, :])
        sc_row = sb.tile([1, M], f32)
        nc.scalar.dma_start(out=sc_row[:], in_=ps_d[:, :])
        ms_row = sb.tile([1, M], f32)
        nc.scalar.dma_start(out=ms_row[:], in_=ms_d[:, :])
        gt_row = sb.tile([1, 4 * N], f32)  # gt-major: [g0(x1 y1 x2 y2), g1(...), ...]
        nc.sync.dma_start(out=gt_row[:], in_=gt_d[:, :])

        # ---- speculative prefetch: out[j] <- mask j (static DMAs) -----
        # gt_boxes are perturbed copies of preds[:N], so gtm[j] == j for
        # most GTs; stream those copies during the match, repair the rest
        # in the post-match cond wave below.
        for j in range(N):
            eng = nc.scalar if j % 2 == 0 else nc.sync
            eng.dma_start(out=outm[j][:, :], in_=slab[j])

        # ---- static constants ----------------------------------------
        iotaF_i = sb.tile([M, M], i32)   # [p, f] = f
        G.iota(iotaF_i[:], pattern=[[1, M]], base=0, channel_multiplier=0)
        iotaP_i = sb.tile([M, 1], i32)   # [p, 0] = p
        G.iota(iotaP_i[:], pattern=[[0, 1]], base=0, channel_multiplier=1)
        iota20b_u = sb.tile([M, N], u32)  # [p, j] = j
        G.iota(iota20b_u[:], pattern=[[1, N]], base=0, channel_multiplier=0,
               allow_small_or_imprecise_dtypes=True)
        iota20b = sb.tile([M, N], f32)
        V.tensor_copy(iota20b[:], iota20b_u[:])

        iotaF = sb.tile([M, M], f32)
        V.tensor_copy(iotaF[:], iotaF_i[:])
        iotaP = sb.tile([M, 1], f32)
        V.tensor_copy(iotaP[:], iotaP_i[:])
        iotaP_bf = sb.tile([M, 1], bf16)
        V.tensor_copy(iotaP_bf[:], iotaP_i[:])

        ones_row = sb.tile([1, M], f32)
        V.memset(ones_row[:], 1.0)
        ones_row_bf = sb.tile([1, M], bf16)
        V.memset(ones_row_bf[:], 1.0)
        ones_col = sb.tile([M, 1], f32)
        V.memset(ones_col[:], 1.0)
        ones_col_bf = sb.tile([M, 1], bf16)
        V.memset(ones_col_bf[:], 1.0)

        # identity (for PE transpose) and "f > p" matrix, from iotas
        ident = sb.tile([M, M], f32)
        V.tensor_scalar(ident[:], iotaF[:], iotaP[:], None, OP.is_equal)
        ltmat = sb.tile([M, M], f32)  # [p, f] = (f > p)
        V.tensor_scalar(ltmat[:], iotaF[:], iotaP[:], None, OP.is_gt)

        # score column [100,1] via PE transpose, and mask_score column
        sc_col = sb.tile([M, 1], f32)
        sccol_ps = ps.tile([M, 1], f32, space="PSUM", tag="ps")
        T.transpose(out=sccol_ps[:], in_=sc_row[:], identity=ident[:1, :1])
        V.tensor_copy(sc_col[:], sccol_ps[:])
        # ---- IoU [100, 20] -------------------------------------------
        # component-major gt row: [x1*20, y1*20, x2*20, y2*20, area2*20]
        crow = sb.tile([1, 5 * N], f32)
        for c in range(4):
            V.tensor_copy(crow[0:1, c * N:(c + 1) * N], gt_row[0:1, c::4])
        aw = sb.tile([1, N], f32)
        V.tensor_sub(aw[:], crow[0:1, 2 * N:3 * N], crow[0:1, 0:N])
        ah = sb.tile([1, N], f32)
        V.tensor_sub(ah[:], crow[0:1, 3 * N:4 * N], crow[0:1, N:2 * N])
        V.tensor_mul(crow[0:1, 4 * N:5 * N], aw[:], ah[:])

        gtb_ps = ps.tile([M, 5 * N], f32, space="PSUM", tag="ps")
        T.matmul(out=gtb_ps[:], lhsT=ones_row[:], rhs=crow[:], start=True, stop=True)
        gx1 = gtb_ps[:, 0:N]
        gy1 = gtb_ps[:, N:2 * N]
        gx2 = gtb_ps[:, 2 * N:3 * N]
        gy2 = gtb_ps[:, 3 * N:4 * N]
        ga = gtb_ps[:, 4 * N:5 * N]

        ltx = sb.tile([M, N], f32)
        V.tensor_scalar(ltx[:], gx1, pb[:, 0:1], None, OP.max)
        lty = sb.tile([M, N], f32)
        V.tensor_scalar(lty[:], gy1, pb[:, 1:2], None, OP.max)
        rbx = sb.tile([M, N], f32)
        V.tensor_scalar(rbx[:], gx2, pb[:, 2:3], None, OP.min)
        rby = sb.tile([M, N], f32)
        V.tensor_scalar(rby[:], gy2, pb[:, 3:4], None, OP.min)
        V.tensor_sub(ltx[:], rbx[:], ltx[:])                       # w
        V.tensor_scalar(ltx[:], ltx[:], 0.0, None, OP.max)
        V.tensor_sub(lty[:], rby[:], lty[:])                       # h
        V.tensor_scalar(lty[:], lty[:], 0.0, None, OP.max)
        inter = sb.tile([M, N], f32)
        V.tensor_mul(inter[:], ltx[:], lty[:])

        ar1a = sb.tile([M, 1], f32)
        V.tensor_sub(ar1a[:], pb[:, 2:3], pb[:, 0:1])
        ar1b = sb.tile([M, 1], f32)
        V.tensor_sub(ar1b[:], pb[:, 3:4], pb[:, 1:2])
        V.tensor_mul(ar1a[:], ar1a[:], ar1b[:])                    # area1 [100,1]

        union = sb.tile([M, N], f32)
        V.tensor_sub(union[:], ga, inter[:])                       # area2 - inter
        V.tensor_scalar(union[:], union[:], ar1a[:], 1e-6, OP.add, OP.max)
        rec = sb.tile([M, N], f32)
        V.reciprocal(rec[:], union[:])
        iou = sb.tile([M, N], f32)
        V.tensor_mul(iou[:], inter[:], rec[:])

        # masked score matrix: iou where (iou >= THR and score ok) else ~ -BIG
        scok = sb.tile([M, 1], f32)
        V.tensor_scalar(scok[:], sc_col[:], SCORE_THR, None, OP.is_ge)
        gate = sb.tile([M, N], f32)
        V.tensor_scalar(gate[:], iou[:], IOU_THR, scok[:], OP.is_ge, OP.mult)
        V.tensor_scalar(gate[:], gate[:], BIG, BIG, OP.mult, OP.subtract)
        masked_s = sb.tile([M, N], f32)
        V.tensor_add(masked_s[:], iou[:], gate[:])
        curmask = sb.tile([M, N], f32)

        # ---- exact priority (stable argsort of -scores) ---------------
        # rank[i] = #{j: s[j] > s[i]} + #{j < i: s[j] == s[i]}; prio = 100 - rank
        srow_ps = ps.tile([M, M], f32, space="PSUM", tag="ps")
        T.matmul(out=srow_ps[:], lhsT=ones_row[:], rhs=sc_row[:], start=True, stop=True)
        a1 = sb.tile([M, M], f32)
        V.tensor_scalar(a1[:], srow_ps[:], sc_col[:], None, OP.is_lt)   # s[f] < s[p]
        aeq = sb.tile([M, M], f32)
        V.tensor_scalar(aeq[:], srow_ps[:], sc_col[:], None, OP.is_equal)
        V.tensor_mul(aeq[:], aeq[:], ltmat[:])          # (s[f]==s[p]) & (f > p)
        V.tensor_add(a1[:], a1[:], aeq[:])              # A[p, f] (p beats f)
        rank_ps = ps.tile([1, M], f32, space="PSUM", tag="ps")    # rank[f] = sum_p A[p, f]
        T.matmul(out=rank_ps[:], lhsT=ones_col[:], rhs=a1[:], start=True, stop=True)
        rank_row = sb.tile([1, M], f32)
        V.tensor_copy(rank_row[:], rank_ps[:])
        rankc_ps = ps.tile([M, 1], f32, space="PSUM", tag="ps")
        T.transpose(out=rankc_ps[:], in_=rank_row[:], identity=ident[:1, :1])
        prio = sb.tile([M, 1], f32)                     # prio = 100 - rank in [1, 100]
        V.tensor_scalar(prio[:], rankc_ps[:], -1.0, float(M), OP.mult, OP.add)

        # ---- match state ---------------------------------------------
        gtm = sb.tile([1, N], f32)
        V.memset(gtm[:], -1.0)
        actp = sb.tile([M, 1], f32)      # prio if active else 0
        V.tensor_copy(actp[:], prio[:])
        maxv8 = sb.tile([M, 8], f32)
        idx8 = sb.tile([M, 8], u32)
        actcnt_i = sb.tile([1, 1], i32)
        combB_i = sb.tile([1, N], i32)

        def peek(src):
            # bids for the NEXT round + deactivate candidates w/o valid bids
            V.max(out=maxv8[:], in_=src[:])
            V.max_index(idx8[:], maxv8[:], src[:])
            V.tensor_scalar(actp[:], maxv8[:, 0:1], NEG_TEST, actp[:],
                            OP.is_gt, OP.mult)

        def commit_round():
            # bid matrix scaled by active priority
            mf = rnd.tile([M, 1], f32, tag="mf")
            V.tensor_copy(mf[:], idx8[:, 0:1])
            bp = rnd.tile([M, N], f32, tag="bp")
            V.tensor_scalar(bp[:], iota20b[:], mf[:], actp[:],
                            OP.is_equal, OP.mult)
            actp2 = rnd.tile([M, 1], f32, tag="actp2")
            V.tensor_mul(actp2[:], actp[:], actp[:])
            # per-GT winner priority broadcast to all partitions
            lb = rnd.tile([M, N], f32, tag="lb")
            G.partition_all_reduce(lb[:], bp[:], channels=M, reduce_op=RED.max)
            # per-candidate: did a higher-priority bidder take my GT?
            lt = rnd.tile([M, N], f32, tag="lt")
            V.tensor_mul(lt[:], lb[:], bp[:])       # actp * winner_prio at own bid
            lo = rnd.tile([M, 1], f32, tag="lo")
            V.tensor_reduce(lo[:], lt[:], AX.X, OP.max)
            loserp = rnd.tile([M, 1], f32, tag="loserp")
            V.tensor_scalar(loserp[:], lo[:], actp2[:], actp[:],
                            OP.is_gt, OP.mult)       # prio of losing bidders
            stopb = rnd.tile([M, 1], f32, tag="stopb")
            G.partition_all_reduce(stopb[:], loserp[:], channels=M,
                                   reduce_op=RED.max)
            # commit candidates strictly above the best loser
            commit = rnd.tile([M, 1], f32, tag="commit")
            V.tensor_tensor(commit[:], actp[:], stopb[:], OP.is_gt)
            V.scalar_tensor_tensor(actp[:], actp[:], stopb[:], actp[:],
                                   OP.is_le, OP.mult)
            wc = rnd.tile([M, N], bf16, tag="wc")
            V.tensor_scalar(wc[:], iota20b[:], mf[:], commit[:],
                            OP.is_equal, OP.mult)
            wrote_ps = ps.tile([1, N], f32, space="PSUM", tag="ps")
            T.matmul(out=wrote_ps[:], lhsT=ones_col_bf[:], rhs=wc[:],
                     start=True, stop=True)
            widx_ps = ps.tile([1, N], f32, space="PSUM", tag="ps")
            T.matmul(out=widx_ps[:], lhsT=iotaP_bf[:], rhs=wc[:],
                     start=True, stop=True)
            # gtm += wrote * (widx - gtm)
            t1 = rnd.tile([1, N], f32, tag="t1")
            V.tensor_sub(t1[:], widx_ps[0:1, :], gtm[:])
            V.tensor_mul(t1[:], t1[:], wrote_ps[0:1, :])
            V.tensor_add(gtm[:], gtm[:], t1[:])
            # rebuild curmask with -BIG on locked GTs (gtm >= 1)
            pen = rnd.tile([1, N], bf16, tag="pen")
            V.tensor_scalar(pen[:], gtm[:], 0.5, -BIG, OP.is_gt, OP.mult)
            penb_ps = ps.tile([M, N], f32, space="PSUM", tag="ps")
            T.matmul(out=penb_ps[:], lhsT=ones_row_bf[:], rhs=pen[:],
                     start=True, stop=True)
            V.tensor_add(curmask[:], masked_s[:], penb_ps[:])

        ck_stack = contextlib.ExitStack()

        def checkpoint(tag):
            acnt_ps = ps.tile([1, 1], f32, space="PSUM", tag="ps")
            T.matmul(out=acnt_ps[:], lhsT=actp[:], rhs=ones_col[:],
                     start=True, stop=True)
            V.tensor_copy(actcnt_i[:], acnt_ps[:])
            regs = nc.alloc_registers(
                f"ck_{tag}", bass.OrderedSet([ET.DVE, ET.PE, ET.Pool]))
            for reg in regs:
                nc.reg_load(reg, actcnt_i[0:1, 0:1])
            ck_stack.enter_context(tc.If(nc.snap(regs) > 0))

        peek(masked_s)
        for gi, g in enumerate(ROUND_PLAN):
            if gi > 0:
                checkpoint(gi)
            for _ in range(g):
                commit_round()
                peek(curmask)
        ck_stack.close()

        # ---- post-match: indices + small outputs ---------------------
        mt = sb.tile([1, N], f32)
        V.tensor_scalar(mt[:], gtm[:], -0.5, None, OP.is_gt)       # matched
        comb = sb.tile([1, N], f32)
        V.tensor_scalar(comb[:], mt[:], -float(M), float(M), OP.mult, OP.add)
        icl = sb.tile([1, N], f32)
        V.tensor_scalar(icl[:], gtm[:], 0.0, None, OP.max)
        V.tensor_add(comb[:], comb[:], icl[:])   # idx (matched) / 100 (unmatched)
        # repair wave: skip GTs where the speculative prefetch was right
        spec_ok = sb.tile([1, N], f32)
        V.tensor_tensor(spec_ok[:], gtm[:], iota20b[0:1, :], OP.is_equal)
        V.scalar_tensor_tensor(comb[:], spec_ok[:], 1000.0, comb[:],
                               OP.mult, OP.add)
        V.tensor_copy(combB_i[:], comb[:])

        mask_wave(combB_i, "B", M + 1)

        # small outputs: scores/flags of matched candidates via one-hot matmul
        ms_col = sb.tile([M, 1], f32)
        mscol_ps = ps.tile([M, 1], f32, space="PSUM", tag="ps")
        T.transpose(out=mscol_ps[:], in_=ms_row[:], identity=ident[:1, :1])
        V.tensor_copy(ms_col[:], mscol_ps[:])
        gtmb_ps = ps.tile([M, N], f32, space="PSUM", tag="ps")
        T.matmul(out=gtmb_ps[:], lhsT=ones_row[:], rhs=gtm[:], start=True, stop=True)
        wsel = sb.tile([M, N], f32)
        V.tensor_scalar(wsel[:], gtmb_ps[:], iotaP[:], None, OP.is_equal)
        sm2 = sb.tile([M, 2], f32)
        V.tensor_copy(sm2[:, 0:1], sc_col[:])
        V.tensor_copy(sm2[:, 1:2], ms_col[:])
        o2_ps = ps.tile([2, N], f32, space="PSUM", tag="ps")
        T.matmul(out=o2_ps[:], lhsT=sm2[:], rhs=wsel[:], start=True, stop=True)
        o2 = sb.tile([2, N], f32)
        V.tensor_copy(o2[:], o2_ps[:])
        nc.sync.dma_start(out=outs[:, :], in_=o2[0:1, :])
        nc.sync.dma_start(out=outf[:, :], in_=o2[1:2, :])

    nc.compile()
    return nc


def _get_nc():
    if "nc" not in _CACHE:
        _CACHE["nc"] = _build()
    return _CACHE["nc"]


def kernel(pred_boxes, pred_scores, mask_score, pred_masks, gt_boxes):
    from concourse.bass_utils import run_bass_kernel_spmd

    nc = _get_nc()

    pred_boxes = np.ascontiguousarray(pred_boxes, dtype=np.float32)
    pred_scores = np.ascontiguousarray(pred_scores, dtype=np.float32).reshape(1, M)
    mask_score = np.ascontiguousarray(mask_score, dtype=np.float32).reshape(1, M)
    gt_boxes = np.ascontiguousarray(gt_boxes, dtype=np.float32).reshape(1, 4 * N)
    pred_masks = np.asarray(pred_masks, dtype=np.float32)

    in_maps = []
    for c in range(NCORES):
        sl = np.empty((M + 1, RS, W), dtype=np.float32)
        sl[:M] = pred_masks[:, c * RS:(c + 1) * RS, :]
        sl[M] = 0.0
        in_maps.append({
            "slab": sl,
            "pred_boxes": pred_boxes,
            "pred_scores": pred_scores,
            "mask_score": mask_score,
            "gt_boxes": gt_boxes,
        })

    res = run_bass_kernel_spmd(nc, in_maps, list(range(NCORES)))
    _CACHE["last_res"] = res
    results = res.results

    gt_bitmasks = np.empty((N, H, W), dtype=np.float32)
    for j in range(N):
        for c in range(NCORES):
            gt_bitmasks[j, c * RS:(c + 1) * RS, :] = results[c][f"out_m_{j}"]
    gt_masks_flags = results[0]["out_flags"].reshape(N)
    out_pred_scores = results[0]["out_scores"].reshape(N)
    return gt_bitmasks, gt_masks_flags, out_pred_scores


# revision 33
# speedup vs baseline: 1.0543x; 1.0318x over previous
"""Trainium2 Bass kernel for nn_BoxSeg_24043226923460 (nms_detection).

Reference computation:
  - pairwise IoU between pred_boxes [100,4] and gt_boxes [20,4]
  - sequential greedy match over candidates in descending-score order
    (faithful to the reference's `gtm[j] <= 0` availability rule)
  - gather: gt_bitmasks[j] = pred_masks[gtm[j]] for matched j else zeros,
    plus matched mask_score / pred_scores vectors.

Device strategy (SPMD over 8 cores):
  - Every core runs the (tiny) match replicated: IoU on 100 partitions,
    exact argsort-by-score priority via a comparison-matrix matmul, and a
    round-based parallel greedy that is provably identical to the
    sequential scan: each round every active candidate bids its argmax
    available GT, and we commit exactly the candidates ranked above the
    highest-priority losing bidder (<= 22 rounds worst case, ~3 typical,
    with a register-If checkpoint after each early round).
  - The heavy data movement is sharded over H: core c owns rows
    [c*100, (c+1)*100) of every mask.  The host appends a zeros page to
    the mask slab (index 100); each of the 20 outputs is its own DRAM
    tensor (separate tensors keep Tile from serializing the copies).
    Because gt_boxes are perturbed copies of preds[:20], gtm[j] == j for
    most GTs, so 20 static speculative DMAs copy mask j -> output j at
    kernel start (hidden under the match), and a post-match repair wave
    of cond-predicated dynamic-offset DRAM->DRAM DMAs rewrites only the
    slots where speculation missed (zeros page for unmatched GTs).
"""

import numpy as np

M, N, H, W = 100, 20, 800, 800
NCORES = 8
RS = H // NCORES  # 100 rows per core
IOU_THR = 0.5
SCORE_THR = 0.05
BIG = 1.0e9
NEG_TEST = -1.0e8  # "is a real iou value" threshold (valid iou >= 0 > NEG_TEST)

# rounds before checkpointing starts=2 (wave A issues after round 2);
# then one checkpoint per early round, groups later.  Total 22.
ROUND_PLAN = [2, 1, 1, 1, 1, 2, 4, 10]

_CACHE = {}


def _build():
    import concourse.bass as bass
    import concourse.mybir as mybir
    from concourse import bacc
    from concourse import bass_isa
    from concourse.tile import TileContext

    f32 = mybir.dt.float32
    bf16 = mybir.dt.bfloat16
    i32 = mybir.dt.int32
    u32 = mybir.dt.uint32
    AX = mybir.AxisListType
    OP = mybir.AluOpType
    ET = mybir.EngineType
    RED = bass_isa.ReduceOp

    nc = bacc.Bacc("TRN2", target_bir_lowering=False, debug=False)

    # ---- DRAM I/O ----------------------------------------------------
    slab = nc.dram_tensor("slab", [M + 1, RS, W], f32, kind="ExternalInput")
    pb_d = nc.dram_tensor("pred_boxes", [M, 4], f32, kind="ExternalInput")
    ps_d = nc.dram_tensor("pred_scores", [1, M], f32, kind="ExternalInput")
    ms_d = nc.dram_tensor("mask_score", [1, M], f32, kind="ExternalInput")
    gt_d = nc.dram_tensor("gt_boxes", [1, 4 * N], f32, kind="ExternalInput")

    outm = [
        nc.dram_tensor(f"out_m_{j}", [RS, W], f32, kind="ExternalOutput")
        for j in range(N)
    ]
    outf = nc.dram_tensor("out_flags", [1, N], f32, kind="ExternalOutput")
    outs = nc.dram_tensor("out_scores", [1, N], f32, kind="ExternalOutput")

    import contextlib

    with TileContext(nc) as tc, contextlib.ExitStack() as pools:
        sb = pools.enter_context(tc.tile_pool(name="sb", bufs=1))
        ps = pools.enter_context(tc.tile_pool(name="ps", bufs=3, space="PSUM"))
        rnd = pools.enter_context(tc.tile_pool(name="rnd", bufs=2))

        V = nc.vector
        T = nc.tensor
        G = nc.gpsimd

        def mask_wave(comb_i_tile, wave, cond_lt):
            """20 dynamic-offset DRAM->DRAM mask copies, split over the two
            HWDGE engines, indices fetched with one multi-register load per
            engine.  Bounds claim M even though skipped values exceed it:
            the cond multiply zeroes the offset before use, and the
            intermediates stay far below 2^31 for every real value."""
            for half, (eng, et) in enumerate(
                    [(nc.scalar, ET.Activation), (nc.sync, ET.SP)]):
                regs = [nc.alloc_register(et, f"dm_{wave}_{half}_{k}")
                        for k in range(N // 2)]
                j0 = half * (N // 2)
                eng.reg_load(regs, comb_i_tile[0:1, j0:j0 + N // 2])
                for k, r in enumerate(regs):
                    j = j0 + k
                    sv = nc.s_assert_within(
                        bass.RuntimeValue(r), 0, M, skip_runtime_assert=True)
                    eng.dma_start(out=outm[j][:, :], in_=slab[sv],
                                  cond=sv < cond_lt, cond_hint=True)

        # ---- load small inputs (spread across DMA engines) -----------
        pb = sb.tile([M, 4], f32)
        nc.sync.dma_start(out=pb[:], in_=pb_d[:, :])
        sc_row = sb.tile([1, M], f32)
        nc.scalar.dma_start(out=sc_row[:], in_=ps_d[:, :])
        ms_row = sb.tile([1, M], f32)
        nc.scalar.dma_start(out=ms_row[:], in_=ms_d[:, :])
        gt_row = sb.tile([1, 4 * N], f32)  # gt-major: [g0(x1 y1 x2 y2), g1(...), ...]
        nc.sync.dma_start(out=gt_row[:], in_=gt_d[:, :])

        # ---- speculative prefetch: out[j] <- mask j (static DMAs) -----
        # gt_boxes are perturbed copies of preds[:N], so gtm[j] == j for
        # most GTs; stream those copies during the match, repair the rest
        # in the post-match cond wave below.
        for j in range(N):
            eng = nc.scalar if j % 2 == 0 else nc.sync
            eng.dma_start(out=outm[j][:, :], in_=slab[j])

        # ---- static constants ----------------------------------------
        iotaF_i = sb.tile([M, M], i32)   # [p, f] = f
        G.iota(iotaF_i[:], pattern=[[1, M]], base=0, channel_multiplier=0)
        iotaP_i = sb.tile([M, 1], i32)   # [p, 0] = p
        G.iota(iotaP_i[:], pattern=[[0, 1]], base=0, channel_multiplier=1)
        iota20b_u = sb.tile([M, N], u32)  # [p, j] = j
        G.iota(iota20b_u[:], pattern=[[1, N]], base=0, channel_multiplier=0,
               allow_small_or_imprecise_dtypes=True)
        iota20b = sb.tile([M, N], f32)
        V.tensor_copy(iota20b[:], iota20b_u[:])

        iotaF = sb.tile([M, M], f32)
        V.tensor_copy(iotaF[:], iotaF_i[:])
        iotaP = sb.tile([M, 1], f32)
        V.tensor_copy(iotaP[:], iotaP_i[:])
        iotaP_bf = sb.tile([M, 1], bf16)
        V.tensor_copy(iotaP_bf[:], iotaP_i[:])

        ones_row = sb.tile([1, M], f32)
        V.memset(ones_row[:], 1.0)
        ones_row_bf = sb.tile([1, M], bf16)
        V.memset(ones_row_bf[:], 1.0)
        ones_col = sb.tile([M, 1], f32)
        V.memset(ones_col[:], 1.0)
        ones_col_bf = sb.tile([M, 1], bf16)
        V.memset(ones_col_bf[:], 1.0)

        # identity (for PE transpose) and "f > p" matrix, from iotas
        ident = sb.tile([M, M], f32)
        V.tensor_scalar(ident[:], iotaF[:], iotaP[:], None, OP.is_equal)
        ident_bf = sb.tile([M, M], bf16)
        V.tensor_copy(ident_bf[:], ident[:])
        ltmat = sb.tile([M, M], f32)  # [p, f] = (f > p)
        V.tensor_scalar(ltmat[:], iotaF[:], iotaP[:], None, OP.is_gt)

        # score column [100,1] via PE transpose, and mask_score column
        sc_col = sb.tile([M, 1], f32)
        sccol_ps = ps.tile([M, 1], f32, space="PSUM", tag="ps")
        T.transpose(out=sccol_ps[:], in_=sc_row[:], identity=ident[:1, :1])
        V.tensor_copy(sc_col[:], sccol_ps[:])
        # ---- IoU [100, 20] -------------------------------------------
        # component-major gt row: [x1*20, y1*20, x2*20, y2*20, area2*20]
        crow = sb.tile([1, 5 * N], f32)
        for c in range(4):
            V.tensor_copy(crow[0:1, c * N:(c + 1) * N], gt_row[0:1, c::4])
        aw = sb.tile([1, N], f32)
        V.tensor_sub(aw[:], crow[0:1, 2 * N:3 * N], crow[0:1, 0:N])
        ah = sb.tile([1, N], f32)
        V.tensor_sub(ah[:], crow[0:1, 3 * N:4 * N], crow[0:1, N:2 * N])
        V.tensor_mul(crow[0:1, 4 * N:5 * N], aw[:], ah[:])

        gtb_ps = ps.tile([M, 5 * N], f32, space="PSUM", tag="ps")
        T.matmul(out=gtb_ps[:], lhsT=ones_row[:], rhs=crow[:], start=True, stop=True)
        gx1 = gtb_ps[:, 0:N]
        gy1 = gtb_ps[:, N:2 * N]
        gx2 = gtb_ps[:, 2 * N:3 * N]
        gy2 = gtb_ps[:, 3 * N:4 * N]
        ga = gtb_ps[:, 4 * N:5 * N]

        ltx = sb.tile([M, N], f32)
        V.tensor_scalar(ltx[:], gx1, pb[:, 0:1], None, OP.max)
        lty = sb.tile([M, N], f32)
        V.tensor_scalar(lty[:], gy1, pb[:, 1:2], None, OP.max)
        rbx = sb.tile([M, N], f32)
        V.tensor_scalar(rbx[:], gx2, pb[:, 2:3], None, OP.min)
        rby = sb.tile([M, N], f32)
        V.tensor_scalar(rby[:], gy2, pb[:, 3:4], None, OP.min)
        V.tensor_sub(ltx[:], rbx[:], ltx[:])                       # w
        V.tensor_scalar(ltx[:], ltx[:], 0.0, None, OP.max)
        V.tensor_sub(lty[:], rby[:], lty[:])                       # h
        V.tensor_scalar(lty[:], lty[:], 0.0, None, OP.max)
        inter = sb.tile([M, N], f32)
        V.tensor_mul(inter[:], ltx[:], lty[:])

        ar1a = sb.tile([M, 1], f32)
        V.tensor_sub(ar1a[:], pb[:, 2:3], pb[:, 0:1])
        ar1b = sb.tile([M, 1], f32)
        V.tensor_sub(ar1b[:], pb[:, 3:4], pb[:, 1:2])
        V.tensor_mul(ar1a[:], ar1a[:], ar1b[:])                    # area1 [100,1]

        union = sb.tile([M, N], f32)
        V.tensor_sub(union[:], ga, inter[:])                       # area2 - inter
        V.tensor_scalar(union[:], union[:], ar1a[:], 1e-6, OP.add, OP.max)
        rec = sb.tile([M, N], f32)
        V.reciprocal(rec[:], union[:])
        iou = sb.tile([M, N], f32)
        V.tensor_mul(iou[:], inter[:], rec[:])

        # masked score matrix: iou where (iou >= THR and score ok) else ~ -BIG
        scok = sb.tile([M, 1], f32)
        V.tensor_scalar(scok[:], sc_col[:], SCORE_THR, None, OP.is_ge)
        gate = sb.tile([M, N], f32)
        V.tensor_scalar(gate[:], iou[:], IOU_THR, scok[:], OP.is_ge, OP.mult)
        V.tensor_scalar(gate[:], gate[:], BIG, BIG, OP.mult, OP.subtract)
        masked_s = sb.tile([M, N], f32)
        V.tensor_add(masked_s[:], iou[:], gate[:])
        curmask = sb.tile([M, N], f32)

        # ---- exact priority (stable argsort of -scores) ---------------
        # rank[i] = #{j: s[j] > s[i]} + #{j < i: s[j] == s[i]}; prio = 100 - rank
        srow_ps = ps.tile([M, M], f32, space="PSUM", tag="ps")
        T.matmul(out=srow_ps[:], lhsT=ones_row[:], rhs=sc_row[:], start=True, stop=True)
        a1 = sb.tile([M, M], f32)
        V.tensor_scalar(a1[:], srow_ps[:], sc_col[:], None, OP.is_lt)   # s[f] < s[p]
        aeq = sb.tile([M, M], f32)
        V.tensor_scalar(aeq[:], srow_ps[:], sc_col[:], None, OP.is_equal)
        V.tensor_mul(aeq[:], aeq[:], ltmat[:])          # (s[f]==s[p]) & (f > p)
        V.tensor_add(a1[:], a1[:], aeq[:])              # A[p, f] (p beats f)
        rank_ps = ps.tile([1, M], f32, space="PSUM", tag="ps")    # rank[f] = sum_p A[p, f]
        T.matmul(out=rank_ps[:], lhsT=ones_col[:], rhs=a1[:], start=True, stop=True)
        rank_row = sb.tile([1, M], f32)
        V.tensor_copy(rank_row[:], rank_ps[:])
        rankc_ps = ps.tile([M, 1], f32, space="PSUM", tag="ps")
        T.transpose(out=rankc_ps[:], in_=rank_row[:], identity=ident[:1, :1])
        prio = sb.tile([M, 1], f32)                     # prio = 100 - rank in [1, 100]
        V.tensor_scalar(prio[:], rankc_ps[:], -1.0, float(M), OP.mult, OP.add)

        # ---- match state ---------------------------------------------
        gtm = sb.tile([1, N], f32)
        V.memset(gtm[:], -1.0)
        actp = sb.tile([M, 1], f32)      # prio if active else 0
        V.tensor_copy(actp[:], prio[:])
        maxv8 = sb.tile([M, 8], f32)
        idx8 = sb.tile([M, 8], u32)
        actcnt_i = sb.tile([1, 1], i32)
        combB_i = sb.tile([1, N], i32)

        def peek(src):
            # bids for the NEXT round + deactivate candidates w/o valid bids
            V.max(out=maxv8[:], in_=src[:])
            V.max_index(idx8[:], maxv8[:], src[:])
            V.tensor_scalar(actp[:], maxv8[:, 0:1], NEG_TEST, actp[:],
                            OP.is_gt, OP.mult)

        def commit_round():
            # bid matrix scaled by active priority
            mf = rnd.tile([M, 1], f32, tag="mf")
            V.tensor_copy(mf[:], idx8[:, 0:1])
            bp = rnd.tile([M, N], bf16, tag="bp")
            V.tensor_scalar(bp[:], iota20b[:], mf[:], actp[:],
                            OP.is_equal, OP.mult)
            # per-GT top-2 bid priorities via PE transpose + vector.max:
            # the highest second-place bid anywhere is the commit stop.
            bpt_ps = ps.tile([N, M], bf16, space="PSUM", tag="ps")
            T.transpose(out=bpt_ps[:], in_=bp[:], identity=ident_bf[:])
            bpt = rnd.tile([N, M], f32, tag="bpt")
            V.tensor_copy(bpt[:], bpt_ps[:])
            win8 = rnd.tile([N, 8], f32, tag="win8")
            V.max(out=win8[:], in_=bpt[:])
            s2t_ps = ps.tile([1, N], f32, space="PSUM", tag="ps")
            T.transpose(out=s2t_ps[:], in_=win8[:, 1:2], identity=ident[:N, :N])
            s2s = rnd.tile([1, N], f32, tag="s2s")
            V.tensor_copy(s2s[:], s2t_ps[:])
            stop = rnd.tile([1, 1], f32, tag="stop")
            V.tensor_reduce(stop[:], s2s[:], AX.X, OP.max)
            stopb_ps = ps.tile([M, 1], f32, space="PSUM", tag="ps")
            T.matmul(out=stopb_ps[:], lhsT=ones_row[:], rhs=stop[:],
                     start=True, stop=True)
            # commit candidates strictly above the best loser
            commit = rnd.tile([M, 1], f32, tag="commit")
            V.tensor_tensor(commit[:], actp[:], stopb_ps[:], OP.is_gt)
            V.scalar_tensor_tensor(actp[:], actp[:], stopb_ps[:], actp[:],
                                   OP.is_le, OP.mult)
            wc = rnd.tile([M, N], bf16, tag="wc")
            V.tensor_scalar(wc[:], iota20b[:], mf[:], commit[:],
                            OP.is_equal, OP.mult)
            wrote_ps = ps.tile([1, N], f32, space="PSUM", tag="ps")
            T.matmul(out=wrote_ps[:], lhsT=ones_col_bf[:], rhs=wc[:],
                     start=True, stop=True)
            widx_ps = ps.tile([1, N], f32, space="PSUM", tag="ps")
            T.matmul(out=widx_ps[:], lhsT=iotaP_bf[:], rhs=wc[:],
                     start=True, stop=True)
            # gtm += wrote * (widx - gtm)
            t1 = rnd.tile([1, N], f32, tag="t1")
            V.tensor_sub(t1[:], widx_ps[0:1, :], gtm[:])
            V.tensor_mul(t1[:], t1[:], wrote_ps[0:1, :])
            V.tensor_add(gtm[:], gtm[:], t1[:])
            # rebuild curmask with -BIG on locked GTs (gtm >= 1)
            pen = rnd.tile([1, N], bf16, tag="pen")
            V.tensor_scalar(pen[:], gtm[:], 0.5, -BIG, OP.is_gt, OP.mult)
            penb_ps = ps.tile([M, N], f32, space="PSUM", tag="ps")
            T.matmul(out=penb_ps[:], lhsT=ones_row_bf[:], rhs=pen[:],
                     start=True, stop=True)
            V.tensor_add(curmask[:], masked_s[:], penb_ps[:])

        ck_stack = contextlib.ExitStack()

        def checkpoint(tag):
            acnt_ps = ps.tile([1, 1], f32, space="PSUM", tag="ps")
            T.matmul(out=acnt_ps[:], lhsT=actp[:], rhs=ones_col[:],
                     start=True, stop=True)
            V.tensor_copy(actcnt_i[:], acnt_ps[:])
            regs = nc.alloc_registers(
                f"ck_{tag}", bass.OrderedSet([ET.DVE, ET.PE]))
            for reg in regs:
                nc.reg_load(reg, actcnt_i[0:1, 0:1])
            ck_stack.enter_context(tc.If(nc.snap(regs) > 0))

        peek(masked_s)
        for gi, g in enumerate(ROUND_PLAN):
            if gi > 0:
                checkpoint(gi)
            for _ in range(g):
                commit_round()
                peek(curmask)
        ck_stack.close()

        # ---- post-match: indices + small outputs ---------------------
        mt = sb.tile([1, N], f32)
        V.tensor_scalar(mt[:], gtm[:], -0.5, None, OP.is_gt)       # matched
        comb = sb.tile([1, N], f32)
        V.tensor_scalar(comb[:], mt[:], -float(M), float(M), OP.mult, OP.add)
        icl = sb.tile([1, N], f32)
        V.tensor_scalar(icl[:], gtm[:], 0.0, None, OP.max)
        V.tensor_add(comb[:], comb[:], icl[:])   # idx (matched) / 100 (unmatched)
        # repair wave: skip GTs where the speculative prefetch was right
        spec_ok = sb.tile([1, N], f32)
        V.tensor_tensor(spec_ok[:], gtm[:], iota20b[0:1, :], OP.is_equal)
        V.scalar_tensor_tensor(comb[:], spec_ok[:], 1000.0, comb[:],
                               OP.mult, OP.add)
        V.tensor_copy(combB_i[:], comb[:])

        mask_wave(combB_i, "B", M + 1)

        # small outputs: scores/flags of matched candidates via one-hot matmul
        ms_col = sb.tile([M, 1], f32)
        mscol_ps = ps.tile([M, 1], f32, space="PSUM", tag="ps")
        T.transpose(out=mscol_ps[:], in_=ms_row[:], identity=ident[:1, :1])
        V.tensor_copy(ms_col[:], mscol_ps[:])
        gtmb_ps = ps.tile([M, N], f32, space="PSUM", tag="ps")
        T.matmul(out=gtmb_ps[:], lhsT=ones_row[:], rhs=gtm[:], start=True, stop=True)
        wsel = sb.tile([M, N], f32)
        V.tensor_scalar(wsel[:], gtmb_ps[:], iotaP[:], None, OP.is_equal)
        sm2 = sb.tile([M, 2], f32)
        V.tensor_copy(sm2[:, 0:1], sc_col[:])
        V.tensor_copy(sm2[:, 1:2], ms_col[:])
        o2_ps = ps.tile([2, N], f32, space="PSUM", tag="ps")
        T.matmul(out=o2_ps[:], lhsT=sm2[:], rhs=wsel[:], start=True, stop=True)
        o2 = sb.tile([2, N], f32)
        V.tensor_copy(o2[:], o2_ps[:])
        nc.sync.dma_start(out=outs[:, :], in_=o2[0:1, :])
        nc.sync.dma_start(out=outf[:, :], in_=o2[1:2, :])

    nc.compile()
    return nc


def _get_nc():
    if "nc" not in _CACHE:
        _CACHE["nc"] = _build()
    return _CACHE["nc"]


def kernel(pred_boxes, pred_scores, mask_score, pred_masks, gt_boxes):
    from concourse.bass_utils import run_bass_kernel_spmd

    nc = _get_nc()

    pred_boxes = np.ascontiguousarray(pred_boxes, dtype=np.float32)
    pred_scores = np.ascontiguousarray(pred_scores, dtype=np.float32).reshape(1, M)
    mask_score = np.ascontiguousarray(mask_score, dtype=np.float32).reshape(1, M)
    gt_boxes = np.ascontiguousarray(gt_boxes, dtype=np.float32).reshape(1, 4 * N)
    pred_masks = np.asarray(pred_masks, dtype=np.float32)

    in_maps = []
    for c in range(NCORES):
        sl = np.empty((M + 1, RS, W), dtype=np.float32)
        sl[:M] = pred_masks[:, c * RS:(c + 1) * RS, :]
        sl[M] = 0.0
        in_maps.append({
            "slab": sl,
            "pred_boxes": pred_boxes,
            "pred_scores": pred_scores,
            "mask_score": mask_score,
            "gt_boxes": gt_boxes,
        })

    res = run_bass_kernel_spmd(nc, in_maps, list(range(NCORES)))
    _CACHE["last_res"] = res
    results = res.results

    gt_bitmasks = np.empty((N, H, W), dtype=np.float32)
    for j in range(N):
        for c in range(NCORES):
            gt_bitmasks[j, c * RS:(c + 1) * RS, :] = results[c][f"out_m_{j}"]
    gt_masks_flags = results[0]["out_flags"].reshape(N)
    out_pred_scores = results[0]["out_scores"].reshape(N)
    return gt_bitmasks, gt_masks_flags, out_pred_scores
